# revision 34
# baseline (speedup 1.0000x reference)
"""Multi-head self-attention (b=4, n=2048, f=1024, h=16) on 8 trn2 NeuronCores.

Sharding: core c -> batch c//2, head-half c%2 (8 heads of 64 dims each).
Each core computes its 8 heads' attention and a partial output projection
(attn_slice @ Wo_rows); host sums the two partials per batch and adds bo.

Device dataflow per core (all matmul operands bf16, PSUM fp32):
  qT/kT  = (x@Wq+bq)^T, (x@Wk+bk)^T  laid out [feat, tok]    (W stationary)
  v      = x@Wv+bv                   laid out [tok, feat]    (xT stationary)
  S1     = [v | 1]            per-head stationaries [tok, 65]
  S0     = e^{-m} * [v | 1]
  logitsT[j, i] = k_j . q_i   (keys on partitions, 2 heads row-packed in PE)
  Etil   = exp(logitsT/32 + m_j)     (ACT bias folds the additive mask for
                                      m_i=1 queries multiplicatively)
  A1/D1  = S1^T @ Etil  (masked numerator + denominator, ones-column trick)
  A0/D0  = S0^T @ Etil  (unmasked variant; e^{-m_j} undoes the bias)
  out_i  = m_i ? A1/D1 : A0/D0   (per-column select via host mask rows)
"""

import sys

sys.path.insert(0, "/opt/trn_rl_repo")

import numpy as np
import ml_dtypes

import concourse.bass as bass
import concourse.bacc as bacc
import concourse.mybir as mybir
import concourse.tile as tile
from concourse import bass_utils

BF16 = mybir.dt.bfloat16
F32 = mybir.dt.float32
NPBF16 = ml_dtypes.bfloat16

B, N, F, H, HD = 4, 2048, 1024, 16, 64
FH = 512          # features per core (8 heads)
NC_ = 8           # cores
NTOKC = N // 128  # 16 token chunks
NIBLK = N // 512  # 4 query blocks
NJ = N // 128     # 16 key chunks
NPAIR = 4         # head pairs per core
EXPFN = mybir.ActivationFunctionType.Exp


def _emit(nc, tc, d, sorted_mode):
    """Emit the whole per-core program under TileContext tc.

    d: dict of dram tensor APs by name.
    sorted_mode: tokens are host-sorted by mask desc, with the 0/1 boundary
    inside query blocks 1..2 — blocks 0 and 3 run a single AV variant.
    """
    consts = tc.alloc_tile_pool(name="consts", bufs=1)
    persist = tc.alloc_tile_pool(name="persist", bufs=1)

    # ---- persistent activations ----------------------------------------
    qT_sb = persist.tile([128, 4 * N], BF16)   # [feat, tok], chunk fc at cols fc*N
    kT_sb = persist.tile([128, 4 * N], BF16)
    s1_sb = persist.tile([128, NJ * 8 * 65], BF16)  # per (jc, head): [v | 1]
    s0_sb = persist.tile([128, NJ * 8 * 65], BF16)  # e^{-m} * [v | 1]
    attnT = persist.tile([128, 4 * N], BF16)   # normalized attn, [feat, tok]

    # ================= phase 1: projections ==============================
    with tc.tile_pool(name="p1sb", bufs=1) as p1sb, \
         tc.tile_pool(name="pkt", bufs=1, space="PSUM") as pkt:
        # xT + Wk loads first (kT matmuls consume them chunk by chunk)
        xT_sb = p1sb.tile([128, 8 * N], BF16)
        wk_sb = p1sb.tile([128, 8 * FH], BF16)
        bqk = consts.tile([128, 8], F32)       # bq chunks (0-3), bk chunks (4-7)
        nc.sync.dma_start(out=bqk, in_=d["bqk"])
        for fc in range(8):
            nc.sync.dma_start(
                out=wk_sb[:, fc * FH:(fc + 1) * FH],
                in_=d["wk"][fc * 128:(fc + 1) * 128, :],
            )
            nc.sync.dma_start(
                out=xT_sb[:, fc * N:(fc + 1) * N],
                in_=d["xT"][fc * 128:(fc + 1) * 128, :],
            )

        # kT: fc-outer over 4 concurrent psum tiles (full PSUM) so the first
        # matmuls start as soon as chunk 0 of xT/Wk lands.
        for grp in range(2):
            pks = [
                pkt.tile([128, 1024], F32, tag=f"pp{t}", name=f"pk{t}")
                for t in range(4)
            ]
            for fc in range(8):
                for t in range(4):
                    fhc, half = grp * 2 + t // 2, t % 2
                    lhsT = wk_sb[:, fc * FH + fhc * 128: fc * FH + (fhc + 1) * 128]
                    for nn in range(2):
                        off = half * 1024 + nn * 512
                        nc.tensor.matmul(
                            pks[t][:, nn * 512:(nn + 1) * 512],
                            lhsT,
                            xT_sb[:, fc * N + off: fc * N + off + 512],
                            start=(fc == 0),
                            stop=(fc == 7),
                        )
            for t in range(4):
                fhc, half = grp * 2 + t // 2, t % 2
                nc.vector.tensor_scalar_add(
                    out=kT_sb[:, fhc * N + half * 1024: fhc * N + half * 1024 + 1024],
                    in0=pks[t][:],
                    scalar1=bqk[:, 4 + fhc: 5 + fhc],
                )

        # remaining loads (emitted after kT matmuls so they queue behind)
        mjb = consts.tile([128, NJ], F32)      # exp bias columns (m per key chunk)
        nc.sync.dma_start(out=mjb, in_=d["mjb"])
        emn = consts.tile([128, NTOKC], F32)   # e^{-m} per token chunk
        nc.sync.dma_start(out=emn, in_=d["emn"])
        mr4 = consts.tile([4, N], F32)         # rows [m, 1-m, m, 1-m]
        nc.sync.dma_start(out=mr4, in_=d["mr4"])
        mrp1 = consts.tile([2, N], F32)        # rows [m, m]
        nc.sync.dma_start(out=mrp1, in_=d["mrp"][0:2, :])
        mrp0 = consts.tile([2, N], F32)        # rows [1-m, 1-m]
        nc.sync.dma_start(out=mrp0, in_=d["mrp"][2:4, :])
        bvb = consts.tile([128, FH], F32)      # bv broadcast over partitions
        nc.sync.dma_start(out=bvb, in_=d["bvb"])
        wo_sb = consts.tile([128, 4 * 1024], BF16)
        for fc in range(4):
            nc.sync.dma_start(
                out=wo_sb[:, fc * 1024:(fc + 1) * 1024],
                in_=d["wo"][fc * 128:(fc + 1) * 128, :],
            )
        w_sb = {}
        for wname in ("wq", "wv"):
            t = p1sb.tile([128, 8 * FH], BF16, tag=wname)
            for fc in range(8):
                nc.sync.dma_start(
                    out=t[:, fc * FH:(fc + 1) * FH],
                    in_=d[wname][fc * 128:(fc + 1) * 128, :],
                )
            w_sb[wname] = t

        # qT (xT is resident by now; fc-inner keeps PSUM small)
        for fhc in range(4):
            for half in range(2):
                pk = pkt.tile(
                    [128, 1024], F32, tag=f"pp{(fhc * 2 + half) % 2}", name="pkq"
                )
                for fc in range(8):
                    lhsT = w_sb["wq"][:, fc * FH + fhc * 128: fc * FH + (fhc + 1) * 128]
                    for nn in range(2):
                        off = half * 1024 + nn * 512
                        nc.tensor.matmul(
                            pk[:, nn * 512:(nn + 1) * 512],
                            lhsT,
                            xT_sb[:, fc * N + off: fc * N + off + 512],
                            start=(fc == 0),
                            stop=(fc == 7),
                        )
                nc.vector.tensor_scalar_add(
                    out=qT_sb[:, fhc * N + half * 1024: fhc * N + half * 1024 + 1024],
                    in0=pk[:],
                    scalar1=bqk[:, fhc: fhc + 1],
                )

        # v: out[tok_chunk 128, fh 512] = xT_chunk^T @ Wv ; then build S1/S0
        for tokc in range(NTOKC):
            pv = pkt.tile([128, FH], F32, tag=f"pp{2 + tokc % 2}", name="pv")
            for fc in range(8):
                nc.tensor.matmul(
                    pv[:],
                    xT_sb[:, fc * N + tokc * 128: fc * N + (tokc + 1) * 128],
                    w_sb["wv"][:, fc * FH:(fc + 1) * FH],
                    start=(fc == 0),
                    stop=(fc == 7),
                )
            base = tokc * 8 * 65
            s1_v = s1_sb[:, base:base + 8 * 65].rearrange("p (h c) -> p h c", h=8)
            s0_v = s0_sb[:, base:base + 8 * 65].rearrange("p (h c) -> p h c", h=8)
            pv_v = pv[:].rearrange("p (h c) -> p h c", h=8)
            bv_v = bvb[:].rearrange("p (h c) -> p h c", h=8)
            # S1 = v + bv (head-strided dest, ones col at c=64)
            nc.vector.tensor_add(out=s1_v[:, :, 0:64], in0=pv_v, in1=bv_v)
            nc.vector.memset(s1_v[:, :, 64:65], 1.0)
            # S0 = e^{-m} * S1
            nc.vector.tensor_scalar_mul(
                out=s0_v[:, :, 0:64],
                in0=s1_v[:, :, 0:64],
                scalar1=emn[:, tokc:tokc + 1],
            )
            emn_b = bass.AP(
                tensor=emn.tensor,
                offset=emn[:, tokc:tokc + 1].offset,
                ap=[emn[:, tokc:tokc + 1].ap[0], [0, 8], [1, 1]],
            )
            nc.vector.tensor_copy(out=s0_v[:, :, 64:65], in_=emn_b)

    # ================= phase 2: attention ================================
    with tc.tile_pool(name="pP", bufs=2, space="PSUM") as pP, \
         tc.tile_pool(name="pacc", bufs=1, space="PSUM") as pacc, \
         tc.tile_pool(name="sexp", bufs=3) as sexp, \
         tc.tile_pool(name="episb", bufs=2) as episb, \
         tc.tile_pool(name="osb", bufs=3) as osb, \
         tc.tile_pool(name="epidr", bufs=2, space="DRAM") as epidr:

        # O-projection micro-ops (one instruction each). In sorted mode they
        # are drained into the pure query blocks' PE slack, using the acc
        # slots the single-variant blocks leave free.
        pending = []

        def o_ops_for_iblk(ib, tags, fin_act=False):
            ops = []
            from itertools import cycle
            tagc = cycle(tags)
            for tokc in range(ib * 4, ib * 4 + 4):
                for half in range(2):
                    st = {}
                    for fc in range(4):
                        def mm(fc=fc, tokc=tokc, half=half, st=st):
                            if fc == 0:
                                st["po"] = pacc.tile(
                                    [128, 512], F32, tag=next(tagc), name="po"
                                )
                            nc.tensor.matmul(
                                st["po"][:],
                                attnT[:, fc * N + tokc * 128: fc * N + (tokc + 1) * 128],
                                wo_sb[:, fc * 1024 + half * 512: fc * 1024 + half * 512 + 512],
                                start=(fc == 0),
                                stop=(fc == 3),
                            )
                        ops.append(mm)

                    def fin(tokc=tokc, half=half, st=st):
                        ot = osb.tile([128, 512], F32, tag="ot", name="ot")
                        if fin_act:
                            # tail runs after all exps: ScalarE is idle and
                            # the DVE queue is busy with the last epilogue
                            nc.scalar.activation(
                                out=ot, in_=st["po"][:],
                                func=mybir.ActivationFunctionType.Copy,
                            )
                        else:
                            nc.vector.tensor_copy(out=ot, in_=st["po"][:])
                        nc.sync.dma_start(
                            out=d["y"][tokc * 128:(tokc + 1) * 128,
                                       half * 512:(half + 1) * 512],
                            in_=ot,
                        )
                    ops.append(fin)
            return ops

        iblk_order = [1, 2, 0, NIBLK - 1] if sorted_mode else list(range(NIBLK))
        for iblk in iblk_order:
            # variant v: 0/2 = masked (A1) for heads A/B, 1/3 = unmasked (A0).
            # With host-sorted tokens, query block 0 is all m_i=1 and block 3
            # all m_i=0, so those need only one AV variant.
            if sorted_mode and iblk == 0:
                active = [0, 2]
            elif sorted_mode and iblk == NIBLK - 1:
                active = [1, 3]
            else:
                active = [0, 1, 2, 3]
            for pair in range(NPAIR):
                accs = {
                    v: pacc.tile([65, 512], F32, tag=f"acc{v}", name=f"acc{v}")
                    for v in active
                }
                def qk(j):
                    P = pP.tile([128, 1024], F32, tag="logits")
                    for hl, tp in ((0, 0), (1, 64)):
                        nc.tensor.matmul(
                            P[:, hl * 512:(hl + 1) * 512],
                            kT_sb[tp:tp + 64, pair * N + j * 128: pair * N + (j + 1) * 128],
                            qT_sb[tp:tp + 64, pair * N + iblk * 512: pair * N + (iblk + 1) * 512],
                            start=True,
                            stop=True,
                            tile_position=(tp, 0),
                        )
                    return P

                # software-pipelined emission, QK two iterations ahead: the PE
                # order per period is [QK(j+2); AV(j)], so QK(j+1) always
                # finishes long before exp(j+1) needs it and the exp chain
                # never waits on a matmul.
                P0 = qk(0)
                P1 = qk(1)
                Ptil = {0: P0, 1: P1}
                for j in range(NJ):
                    S = sexp.tile([128, 1024], BF16, tag="etil")
                    nc.scalar.activation(
                        out=S[:], in_=Ptil.pop(j), func=EXPFN,
                        bias=mjb[:, j:j + 1], scale=1.0 / 32.0,
                    )
                    if j + 2 < NJ:
                        Ptil[j + 2] = qk(j + 2)
                    for hl in range(2):
                        hcore = 2 * pair + hl
                        soff = j * 8 * 65 + hcore * 65
                        rhs = S[:, hl * 512:(hl + 1) * 512]
                        for v, s_sb in ((2 * hl, s1_sb), (2 * hl + 1, s0_sb)):
                            if v in accs:
                                nc.tensor.matmul(
                                    accs[v][:], s_sb[:, soff:soff + 65], rhs,
                                    start=(j == 0), stop=(j == NJ - 1),
                                )
                    # pure blocks have PE slack and 2 free acc slots: drain
                    # O-projection micro-ops for already-finished blocks.
                    if len(active) == 2:
                        for _ in range(2):
                            if pending:
                                pending.pop(0)()

                # ---- epilogue: select + normalize -----------------------
                na = len(active)
                asb = {}
                for v in active:
                    t = episb.tile([65, 512], F32, tag=f"asb{v}", name=f"asb{v}")
                    nc.vector.tensor_copy(out=t, in_=accs[v][:])
                    asb[v] = t
                rin = episb.tile([4, 512], F32, tag="rin")
                for k, v in enumerate(active):
                    nc.sync.dma_start(out=rin[k:k + 1, :], in_=asb[v][64:65, :])
                rsc = episb.tile([4, 512], F32, tag="rsc")
                nc.vector.reciprocal_approx_fast(
                    out=rsc[0:na, :], in_=rin[0:na, :]
                )
                # mask rows matching `active`: dual -> [m,1-m,m,1-m]; pure
                # blocks -> [m,m] / [1-m,1-m] (from mrp).
                ib = iblk * 512
                if na == 4:
                    mrow = mr4[:, ib:ib + 512]
                else:
                    mrow = (mrp1 if active[0] == 0 else mrp0)[:, ib:ib + 512]
                nc.vector.tensor_mul(
                    out=rsc[0:na, :], in0=rsc[0:na, :], in1=mrow
                )
                stg2 = epidr.tile([4, 512], F32, tag="stg2")
                nc.sync.dma_start(out=stg2[0:na, :], in_=rsc[0:na, :])
                rball = episb.tile([64, 4 * 512], F32, tag="rball")
                nc.sync.dma_start(
                    out=rball[:, 0:na * 512],
                    in_=bass.AP(tensor=stg2.tensor, offset=stg2.offset,
                                ap=[[0, 64], [512, na], [1, 512]]),
                )
                rb = {
                    v: rball[:, k * 512:(k + 1) * 512]
                    for k, v in enumerate(active)
                }
                for hl in range(2):
                    dstc = pair * N + iblk * 512
                    v1, v0 = 2 * hl, 2 * hl + 1
                    if na == 4:
                        t1 = episb.tile([64, 512], F32, tag="ept1")
                        t2 = episb.tile([64, 512], F32, tag="ept2")
                        nc.vector.tensor_mul(out=t1, in0=asb[v1][0:64, :], in1=rb[v1])
                        nc.vector.tensor_mul(out=t2, in0=asb[v0][0:64, :], in1=rb[v0])
                        if hl == 0:
                            nc.vector.tensor_add(
                                out=attnT[0:64, dstc:dstc + 512], in0=t1, in1=t2
                            )
                        else:
                            t3 = episb.tile([64, 512], BF16, tag="ept3")
                            nc.vector.tensor_add(out=t3, in0=t1, in1=t2)
                            nc.sync.dma_start(
                                out=attnT[64:128, dstc:dstc + 512], in_=t3
                            )
                    else:
                        vv = v1 if v1 in asb else v0
                        if hl == 0:
                            nc.vector.tensor_mul(
                                out=attnT[0:64, dstc:dstc + 512],
                                in0=asb[vv][0:64, :], in1=rb[vv],
                            )
                        else:
                            t3 = episb.tile([64, 512], BF16, tag="ept3")
                            nc.vector.tensor_mul(out=t3, in0=asb[vv][0:64, :], in1=rb[vv])
                            nc.sync.dma_start(
                                out=attnT[64:128, dstc:dstc + 512], in_=t3
                            )

            # queue this block's O-projection. Blocks 1/2/0 drain inside the
            # pure blocks (0 and 3) on free acc slots; block 3 drains at the
            # tail on the slots block 0 used.
            if sorted_mode:
                tags = ("acc1", "acc3") if iblk in (1, 2) else ("acc0", "acc2")
                pending.extend(
                    o_ops_for_iblk(iblk, tags, fin_act=(iblk == NIBLK - 1))
                )
            else:
                pending.extend(o_ops_for_iblk(iblk, ("acc0", "acc2"), fin_act=True))

        # ===== tail: drain remaining O-projection ops ======================
        while pending:
            pending.pop(0)()

    persist.release()
    consts.release()


_CACHE = {}


def build_program(variant="sorted"):
    if variant in _CACHE:
        return _CACHE[variant]
    nc = bacc.Bacc("TRN2", target_bir_lowering=False, debug=False)
    d = {}
    d["xT"] = nc.dram_tensor("xT", (F, N), BF16, kind="ExternalInput").ap()
    d["wq"] = nc.dram_tensor("wq", (F, FH), BF16, kind="ExternalInput").ap()
    d["wk"] = nc.dram_tensor("wk", (F, FH), BF16, kind="ExternalInput").ap()
    d["wv"] = nc.dram_tensor("wv", (F, FH), BF16, kind="ExternalInput").ap()
    d["wo"] = nc.dram_tensor("wo", (FH, F), BF16, kind="ExternalInput").ap()
    d["bqk"] = nc.dram_tensor("bqk", (128, 8), F32, kind="ExternalInput").ap()
    d["bvb"] = nc.dram_tensor("bvb", (128, FH), F32, kind="ExternalInput").ap()
    d["mjb"] = nc.dram_tensor("mjb", (128, NJ), F32, kind="ExternalInput").ap()
    d["emn"] = nc.dram_tensor("emn", (128, NTOKC), F32, kind="ExternalInput").ap()
    d["mr4"] = nc.dram_tensor("mr4", (4, N), F32, kind="ExternalInput").ap()
    d["mrp"] = nc.dram_tensor("mrp", (4, N), F32, kind="ExternalInput").ap()
    d["y"] = nc.dram_tensor("y", (N, F), F32, kind="ExternalOutput").ap()
    with tile.TileContext(nc) as tc:
        _emit(nc, tc, d, sorted_mode=(variant == "sorted"))
    nc.compile()
    _CACHE[variant] = nc
    return nc


def make_in_maps(x, inputs_mask, Wq, bq, Wk, bk, Wv, bv, Wo, bo,
                 sorted_mode=True):
    """Host-side shard prep. All args np.float32/int32 full tensors.

    sorted_mode: per batch, tokens are permuted so mask=1 tokens come first
    (attention is permutation-equivariant when q/k/v share the permutation);
    returns the per-batch permutations for un-permuting the output.
    """
    in_maps = []
    m_all = inputs_mask.astype(np.float32)
    perms = []
    for b in range(B):
        if sorted_mode:
            perms.append(np.argsort(-m_all[b], kind="stable"))
        else:
            perms.append(np.arange(N))
    for c in range(NC_):
        b, hh = c // 2, c % 2
        cs = slice(hh * FH, (hh + 1) * FH)
        m = m_all[b][perms[b]]
        xb = x[b][perms[b]]
        im = {
            "xT": np.ascontiguousarray(xb.T).astype(NPBF16),
            "wq": Wq[:, cs].astype(NPBF16),
            "wk": Wk[:, cs].astype(NPBF16),
            "wv": Wv[:, cs].astype(NPBF16),
            "wo": np.ascontiguousarray(Wo[cs, :]).astype(NPBF16),
            "bqk": np.stack(
                [bq[cs].reshape(4, 128), bk[cs].reshape(4, 128)], axis=0
            ).reshape(8, 128).T.astype(np.float32).copy(),
            "bvb": np.broadcast_to(bv[cs], (128, FH)).astype(np.float32).copy(),
            "mjb": m.reshape(NJ, 128).T.astype(np.float32).copy(),
            "emn": np.exp(-m).reshape(NTOKC, 128).T.astype(np.float32).copy(),
            "mr4": np.stack([m, 1.0 - m, m, 1.0 - m]).astype(np.float32).copy(),
            "mrp": np.stack([m, m, 1.0 - m, 1.0 - m]).astype(np.float32).copy(),
        }
        in_maps.append(im)
    return in_maps, perms


def kernel(x, inputs_mask, Wq, bq, Wk, bk, Wv, bv, Wo, bo):
    x = np.asarray(x, dtype=np.float32)
    inputs_mask = np.asarray(inputs_mask)
    Wq, bq = np.asarray(Wq, np.float32), np.asarray(bq, np.float32)
    Wk, bk = np.asarray(Wk, np.float32), np.asarray(bk, np.float32)
    Wv, bv = np.asarray(Wv, np.float32), np.asarray(bv, np.float32)
    Wo, bo = np.asarray(Wo, np.float32), np.asarray(bo, np.float32)

    # sorted mode requires the mask-1 count per batch to land inside query
    # blocks 1..2 (always true for ~Bernoulli(0.5) masks); fall back to the
    # static dual-pass program otherwise.
    c1 = inputs_mask.astype(np.int64).sum(axis=1)
    sorted_mode = bool(np.all((c1 >= 512) & (c1 <= 3 * 512)))
    nc = build_program("sorted" if sorted_mode else "dual")
    in_maps, perms = make_in_maps(
        x, inputs_mask, Wq, bq, Wk, bk, Wv, bv, Wo, bo, sorted_mode=sorted_mode
    )
    res = bass_utils.run_bass_kernel_spmd(nc, in_maps, core_ids=list(range(NC_)))
    out = np.empty((B, N, F), dtype=np.float32)
    for b in range(B):
        out[b][perms[b]] = (
            res.results[2 * b]["y"] + res.results[2 * b + 1]["y"] + bo
        )
    return out


# revision 38
# speedup vs baseline: 1.0045x; 1.0045x over previous
"""Multi-head self-attention (b=4, n=2048, f=1024, h=16) on 8 trn2 NeuronCores.

Sharding: core c -> batch c//2, head-half c%2 (8 heads of 64 dims each).
Each core computes its 8 heads' attention and a partial output projection
(attn_slice @ Wo_rows); host sums the two partials per batch and adds bo.

Device dataflow per core (all matmul operands bf16, PSUM fp32):
  qT/kT  = (x@Wq+bq)^T, (x@Wk+bk)^T  laid out [feat, tok]    (W stationary)
  v      = x@Wv+bv                   laid out [tok, feat]    (xT stationary)
  S1     = [v | 1]            per-head stationaries [tok, 65]
  S0     = e^{-m} * [v | 1]
  logitsT[j, i] = k_j . q_i   (keys on partitions, 2 heads row-packed in PE)
  Etil   = exp(logitsT/32 + m_j)     (ACT bias folds the additive mask for
                                      m_i=1 queries multiplicatively)
  A1/D1  = S1^T @ Etil  (masked numerator + denominator, ones-column trick)
  A0/D0  = S0^T @ Etil  (unmasked variant; e^{-m_j} undoes the bias)
  out_i  = m_i ? A1/D1 : A0/D0   (per-column select via host mask rows)
"""

import sys

sys.path.insert(0, "/opt/trn_rl_repo")

import numpy as np
import ml_dtypes

import concourse.bass as bass
import concourse.bacc as bacc
import concourse.mybir as mybir
import concourse.tile as tile
from concourse import bass_utils

BF16 = mybir.dt.bfloat16
F32 = mybir.dt.float32
NPBF16 = ml_dtypes.bfloat16

B, N, F, H, HD = 4, 2048, 1024, 16, 64
FH = 512          # features per core (8 heads)
NC_ = 8           # cores
NTOKC = N // 128  # 16 token chunks
NIBLK = N // 512  # 4 query blocks
NJ = N // 128     # 16 key chunks
NPAIR = 4         # head pairs per core
EXPFN = mybir.ActivationFunctionType.Exp


def _emit(nc, tc, d, sorted_mode):
    """Emit the whole per-core program under TileContext tc.

    d: dict of dram tensor APs by name.
    sorted_mode: tokens are host-sorted by mask desc, with the 0/1 boundary
    inside query blocks 1..2 — blocks 0 and 3 run a single AV variant.
    """
    consts = tc.alloc_tile_pool(name="consts", bufs=1)
    persist = tc.alloc_tile_pool(name="persist", bufs=1)

    # ---- persistent activations ----------------------------------------
    qT_sb = persist.tile([128, 4 * N], BF16)   # [feat, tok], chunk fc at cols fc*N
    kT_sb = persist.tile([128, 4 * N], BF16)
    s1_sb = persist.tile([128, NJ * 8 * 65], BF16)  # per (jc, head): [v | 1]
    s0_sb = persist.tile([128, NJ * 8 * 65], BF16)  # e^{-m} * [v | 1]
    attnT = persist.tile([128, 4 * N], BF16)   # normalized attn, [feat, tok]

    # ================= phase 1: projections ==============================
    with tc.tile_pool(name="p1sb", bufs=1) as p1sb, \
         tc.tile_pool(name="pkt", bufs=1, space="PSUM") as pkt:
        # xT + Wk loads first (kT matmuls consume them chunk by chunk)
        xT_sb = p1sb.tile([128, 8 * N], BF16)
        wk_sb = p1sb.tile([128, 8 * FH], BF16)
        bqk = consts.tile([128, 8], F32)       # bq chunks (0-3), bk chunks (4-7)
        nc.sync.dma_start(out=bqk, in_=d["bqk"])
        for fc in range(8):
            nc.sync.dma_start(
                out=wk_sb[:, fc * FH:(fc + 1) * FH],
                in_=d["wk"][fc * 128:(fc + 1) * 128, :],
            )
            nc.sync.dma_start(
                out=xT_sb[:, fc * N:(fc + 1) * N],
                in_=d["xT"][fc * 128:(fc + 1) * 128, :],
            )

        # kT: fc-outer over 4 concurrent psum tiles (full PSUM) so the first
        # matmuls start as soon as chunk 0 of xT/Wk lands.
        for grp in range(2):
            pks = [
                pkt.tile([128, 1024], F32, tag=f"pp{t}", name=f"pk{t}")
                for t in range(4)
            ]
            for fc in range(8):
                for t in range(4):
                    fhc, half = grp * 2 + t // 2, t % 2
                    lhsT = wk_sb[:, fc * FH + fhc * 128: fc * FH + (fhc + 1) * 128]
                    for nn in range(2):
                        off = half * 1024 + nn * 512
                        nc.tensor.matmul(
                            pks[t][:, nn * 512:(nn + 1) * 512],
                            lhsT,
                            xT_sb[:, fc * N + off: fc * N + off + 512],
                            start=(fc == 0),
                            stop=(fc == 7),
                        )
            for t in range(4):
                fhc, half = grp * 2 + t // 2, t % 2
                nc.vector.tensor_scalar_add(
                    out=kT_sb[:, fhc * N + half * 1024: fhc * N + half * 1024 + 1024],
                    in0=pks[t][:],
                    scalar1=bqk[:, 4 + fhc: 5 + fhc],
                )

        # remaining loads (emitted after kT matmuls so they queue behind)
        mjb = consts.tile([128, NJ], F32)      # exp bias columns (m per key chunk)
        nc.sync.dma_start(out=mjb, in_=d["mjb"])
        emn = consts.tile([128, NTOKC], F32)   # e^{-m} per token chunk
        nc.sync.dma_start(out=emn, in_=d["emn"])
        mr4 = consts.tile([4, N], F32)         # rows [m, 1-m, m, 1-m]
        nc.sync.dma_start(out=mr4, in_=d["mr4"])
        mrp1 = consts.tile([2, N], F32)        # rows [m, m]
        nc.sync.dma_start(out=mrp1, in_=d["mrp"][0:2, :])
        mrp0 = consts.tile([2, N], F32)        # rows [1-m, 1-m]
        nc.sync.dma_start(out=mrp0, in_=d["mrp"][2:4, :])
        bvb = consts.tile([128, FH], F32)      # bv broadcast over partitions
        nc.sync.dma_start(out=bvb, in_=d["bvb"])
        wo_sb = consts.tile([128, 4 * 1024], BF16)
        for fc in range(4):
            nc.sync.dma_start(
                out=wo_sb[:, fc * 1024:(fc + 1) * 1024],
                in_=d["wo"][fc * 128:(fc + 1) * 128, :],
            )
        w_sb = {}
        for wname in ("wq", "wv"):
            t = p1sb.tile([128, 8 * FH], BF16, tag=wname)
            for fc in range(8):
                nc.sync.dma_start(
                    out=t[:, fc * FH:(fc + 1) * FH],
                    in_=d[wname][fc * 128:(fc + 1) * 128, :],
                )
            w_sb[wname] = t

        # qT (xT is resident by now; fc-inner keeps PSUM small)
        for fhc in range(4):
            for half in range(2):
                pk = pkt.tile(
                    [128, 1024], F32, tag=f"pp{(fhc * 2 + half) % 2}", name="pkq"
                )
                for fc in range(8):
                    lhsT = w_sb["wq"][:, fc * FH + fhc * 128: fc * FH + (fhc + 1) * 128]
                    for nn in range(2):
                        off = half * 1024 + nn * 512
                        nc.tensor.matmul(
                            pk[:, nn * 512:(nn + 1) * 512],
                            lhsT,
                            xT_sb[:, fc * N + off: fc * N + off + 512],
                            start=(fc == 0),
                            stop=(fc == 7),
                        )
                nc.vector.tensor_scalar_add(
                    out=qT_sb[:, fhc * N + half * 1024: fhc * N + half * 1024 + 1024],
                    in0=pk[:],
                    scalar1=bqk[:, fhc: fhc + 1],
                )

        # v: out[tok_chunk 128, fh 512] = xT_chunk^T @ Wv ; then build S1/S0
        for tokc in range(NTOKC):
            pv = pkt.tile([128, FH], F32, tag=f"pp{2 + tokc % 2}", name="pv")
            for fc in range(8):
                nc.tensor.matmul(
                    pv[:],
                    xT_sb[:, fc * N + tokc * 128: fc * N + (tokc + 1) * 128],
                    w_sb["wv"][:, fc * FH:(fc + 1) * FH],
                    start=(fc == 0),
                    stop=(fc == 7),
                )
            base = tokc * 8 * 65
            s1_v = s1_sb[:, base:base + 8 * 65].rearrange("p (h c) -> p h c", h=8)
            s0_v = s0_sb[:, base:base + 8 * 65].rearrange("p (h c) -> p h c", h=8)
            pv_v = pv[:].rearrange("p (h c) -> p h c", h=8)
            bv_v = bvb[:].rearrange("p (h c) -> p h c", h=8)
            # S1 = v + bv (head-strided dest, ones col at c=64)
            nc.vector.tensor_add(out=s1_v[:, :, 0:64], in0=pv_v, in1=bv_v)
            nc.vector.memset(s1_v[:, :, 64:65], 1.0)
            # S0 = e^{-m} * S1
            nc.vector.tensor_scalar_mul(
                out=s0_v[:, :, 0:64],
                in0=s1_v[:, :, 0:64],
                scalar1=emn[:, tokc:tokc + 1],
            )
            emn_b = bass.AP(
                tensor=emn.tensor,
                offset=emn[:, tokc:tokc + 1].offset,
                ap=[emn[:, tokc:tokc + 1].ap[0], [0, 8], [1, 1]],
            )
            nc.vector.tensor_copy(out=s0_v[:, :, 64:65], in_=emn_b)

    # ================= phase 2: attention ================================
    with tc.tile_pool(name="pP", bufs=2, space="PSUM") as pP, \
         tc.tile_pool(name="pacc", bufs=1, space="PSUM") as pacc, \
         tc.tile_pool(name="sexp", bufs=3) as sexp, \
         tc.tile_pool(name="episb", bufs=2) as episb, \
         tc.tile_pool(name="osb", bufs=3) as osb, \
         tc.tile_pool(name="epidr", bufs=2, space="DRAM") as epidr:

        # O-projection micro-ops (one instruction each). In sorted mode they
        # are drained into the pure query blocks' PE slack, using the acc
        # slots the single-variant blocks leave free.
        pending = []

        def o_ops_for_iblk(ib, tags, fin_act=False):
            ops = []
            from itertools import cycle
            tagc = cycle(tags)
            for tokc in range(ib * 4, ib * 4 + 4):
                for half in range(2):
                    st = {}
                    for fc in range(4):
                        def mm(fc=fc, tokc=tokc, half=half, st=st):
                            if fc == 0:
                                st["po"] = pacc.tile(
                                    [128, 512], F32, tag=next(tagc), name="po"
                                )
                            nc.tensor.matmul(
                                st["po"][:],
                                attnT[:, fc * N + tokc * 128: fc * N + (tokc + 1) * 128],
                                wo_sb[:, fc * 1024 + half * 512: fc * 1024 + half * 512 + 512],
                                start=(fc == 0),
                                stop=(fc == 3),
                            )
                        ops.append(mm)

                    def fin(tokc=tokc, half=half, st=st):
                        ot = osb.tile([128, 512], F32, tag="ot", name="ot")
                        if fin_act:
                            # tail runs after all exps: ScalarE is idle and
                            # the DVE queue is busy with the last epilogue
                            nc.scalar.activation(
                                out=ot, in_=st["po"][:],
                                func=mybir.ActivationFunctionType.Copy,
                            )
                        else:
                            nc.vector.tensor_copy(out=ot, in_=st["po"][:])
                        nc.sync.dma_start(
                            out=d["y"][tokc * 128:(tokc + 1) * 128,
                                       half * 512:(half + 1) * 512],
                            in_=ot,
                        )
                    ops.append(fin)
            return ops

        iblk_order = [1, 2, 0, NIBLK - 1] if sorted_mode else list(range(NIBLK))
        for iblk in iblk_order:
            # variant v: 0/2 = masked (A1) for heads A/B, 1/3 = unmasked (A0).
            # With host-sorted tokens, query block 0 is all m_i=1 and block 3
            # all m_i=0, so those need only one AV variant.
            if sorted_mode and iblk == 0:
                active = [0, 2]
            elif sorted_mode and iblk == NIBLK - 1:
                active = [1, 3]
            else:
                active = [0, 1, 2, 3]
            for pair in range(NPAIR):
                accs = {
                    v: pacc.tile([65, 512], F32, tag=f"acc{v}", name=f"acc{v}")
                    for v in active
                }
                def qk(j):
                    P = pP.tile([128, 1024], F32, tag="logits")
                    for hl, tp in ((0, 0), (1, 64)):
                        nc.tensor.matmul(
                            P[:, hl * 512:(hl + 1) * 512],
                            kT_sb[tp:tp + 64, pair * N + j * 128: pair * N + (j + 1) * 128],
                            qT_sb[tp:tp + 64, pair * N + iblk * 512: pair * N + (iblk + 1) * 512],
                            start=True,
                            stop=True,
                            tile_position=(tp, 0),
                        )
                    return P

                # software-pipelined emission, QK two iterations ahead: the PE
                # order per period is [QK(j+2); AV(j)], so QK(j+1) always
                # finishes long before exp(j+1) needs it and the exp chain
                # never waits on a matmul.
                P0 = qk(0)
                P1 = qk(1)
                Ptil = {0: P0, 1: P1}
                for j in range(NJ):
                    S = sexp.tile([128, 1024], BF16, tag="etil")
                    nc.scalar.activation(
                        out=S[:], in_=Ptil.pop(j), func=EXPFN,
                        bias=mjb[:, j:j + 1], scale=1.0 / 32.0,
                    )
                    if j + 2 < NJ:
                        Ptil[j + 2] = qk(j + 2)
                    for hl in range(2):
                        hcore = 2 * pair + hl
                        soff = j * 8 * 65 + hcore * 65
                        rhs = S[:, hl * 512:(hl + 1) * 512]
                        for v, s_sb in ((2 * hl, s1_sb), (2 * hl + 1, s0_sb)):
                            if v in accs:
                                nc.tensor.matmul(
                                    accs[v][:], s_sb[:, soff:soff + 65], rhs,
                                    start=(j == 0), stop=(j == NJ - 1),
                                )
                    # pure blocks have PE slack and 2 free acc slots: drain
                    # O-projection micro-ops for already-finished blocks.
                    if len(active) == 2:
                        for _ in range(2):
                            if pending:
                                pending.pop(0)()

                # ---- epilogue: select + normalize -----------------------
                na = len(active)
                asb = {}
                for v in active:
                    t = episb.tile([65, 512], F32, tag=f"asb{v}", name=f"asb{v}")
                    nc.vector.tensor_copy(out=t, in_=accs[v][:])
                    asb[v] = t
                rin = episb.tile([4, 512], F32, tag="rin")
                for k, v in enumerate(active):
                    nc.sync.dma_start(out=rin[k:k + 1, :], in_=asb[v][64:65, :])
                rsc = episb.tile([4, 512], F32, tag="rsc")
                nc.vector.reciprocal_approx_fast(
                    out=rsc[0:na, :], in_=rin[0:na, :]
                )
                # mask rows matching `active`: dual -> [m,1-m,m,1-m]; pure
                # blocks -> [m,m] / [1-m,1-m] (from mrp).
                ib = iblk * 512
                if na == 4:
                    mrow = mr4[:, ib:ib + 512]
                else:
                    mrow = (mrp1 if active[0] == 0 else mrp0)[:, ib:ib + 512]
                nc.vector.tensor_mul(
                    out=rsc[0:na, :], in0=rsc[0:na, :], in1=mrow
                )
                stg2 = epidr.tile([4, 512], F32, tag="stg2")
                nc.sync.dma_start(out=stg2[0:na, :], in_=rsc[0:na, :])
                rball = episb.tile([64, 4 * 512], F32, tag="rball")
                nc.sync.dma_start(
                    out=rball[:, 0:na * 512],
                    in_=bass.AP(tensor=stg2.tensor, offset=stg2.offset,
                                ap=[[0, 64], [512, na], [1, 512]]),
                )
                rb = {
                    v: rball[:, k * 512:(k + 1) * 512]
                    for k, v in enumerate(active)
                }
                for hl in range(2):
                    dstc = pair * N + iblk * 512
                    v1, v0 = 2 * hl, 2 * hl + 1
                    if na == 4:
                        t1 = episb.tile([64, 512], F32, tag="ept1")
                        t2 = episb.tile([64, 512], F32, tag="ept2")
                        nc.vector.tensor_mul(out=t1, in0=asb[v1][0:64, :], in1=rb[v1])
                        nc.vector.tensor_mul(out=t2, in0=asb[v0][0:64, :], in1=rb[v0])
                        if hl == 0:
                            nc.vector.tensor_add(
                                out=attnT[0:64, dstc:dstc + 512], in0=t1, in1=t2
                            )
                        else:
                            t3 = episb.tile([64, 512], BF16, tag="ept3")
                            nc.vector.tensor_add(out=t3, in0=t1, in1=t2)
                            nc.sync.dma_start(
                                out=attnT[64:128, dstc:dstc + 512], in_=t3
                            )
                    else:
                        vv = v1 if v1 in asb else v0
                        if hl == 0:
                            nc.vector.tensor_mul(
                                out=attnT[0:64, dstc:dstc + 512],
                                in0=asb[vv][0:64, :], in1=rb[vv],
                            )
                        else:
                            t3 = episb.tile([64, 512], BF16, tag="ept3")
                            nc.vector.tensor_mul(out=t3, in0=asb[vv][0:64, :], in1=rb[vv])
                            nc.sync.dma_start(
                                out=attnT[64:128, dstc:dstc + 512], in_=t3
                            )

            # queue this block's O-projection. Blocks 1/2/0 drain inside the
            # pure blocks (0 and 3) on free acc slots; block 3 drains at the
            # tail on the slots block 0 used.
            if sorted_mode:
                tags = ("acc1", "acc3") if iblk in (1, 2) else ("acc0", "acc2")
                pending.extend(
                    o_ops_for_iblk(iblk, tags, fin_act=(iblk == NIBLK - 1))
                )
            else:
                pending.extend(o_ops_for_iblk(iblk, ("acc0", "acc2"), fin_act=True))

        # ===== tail: drain remaining O-projection ops ======================
        if sorted_mode:
            # The last block's O-proj waits ~13us on its epilogue chain; PE
            # would go idle past the HAM MID window and re-throttle to
            # 1.2GHz. Keep it warm with dependency-free filler matmuls that
            # occupy exactly that window.
            warm = pacc.tile([128, 512], F32, tag="acc1", name="warm")
            for _ in range(50):
                nc.tensor.matmul(
                    warm[:], wo_sb[:, 0:128], wo_sb[:, 0:512],
                    start=True, stop=True,
                )
        while pending:
            pending.pop(0)()

    persist.release()
    consts.release()


_CACHE = {}


def build_program(variant="sorted"):
    if variant in _CACHE:
        return _CACHE[variant]
    nc = bacc.Bacc("TRN2", target_bir_lowering=False, debug=False)
    d = {}
    d["xT"] = nc.dram_tensor("xT", (F, N), BF16, kind="ExternalInput").ap()
    d["wq"] = nc.dram_tensor("wq", (F, FH), BF16, kind="ExternalInput").ap()
    d["wk"] = nc.dram_tensor("wk", (F, FH), BF16, kind="ExternalInput").ap()
    d["wv"] = nc.dram_tensor("wv", (F, FH), BF16, kind="ExternalInput").ap()
    d["wo"] = nc.dram_tensor("wo", (FH, F), BF16, kind="ExternalInput").ap()
    d["bqk"] = nc.dram_tensor("bqk", (128, 8), F32, kind="ExternalInput").ap()
    d["bvb"] = nc.dram_tensor("bvb", (128, FH), F32, kind="ExternalInput").ap()
    d["mjb"] = nc.dram_tensor("mjb", (128, NJ), F32, kind="ExternalInput").ap()
    d["emn"] = nc.dram_tensor("emn", (128, NTOKC), F32, kind="ExternalInput").ap()
    d["mr4"] = nc.dram_tensor("mr4", (4, N), F32, kind="ExternalInput").ap()
    d["mrp"] = nc.dram_tensor("mrp", (4, N), F32, kind="ExternalInput").ap()
    d["y"] = nc.dram_tensor("y", (N, F), F32, kind="ExternalOutput").ap()
    with tile.TileContext(nc) as tc:
        _emit(nc, tc, d, sorted_mode=(variant == "sorted"))
    nc.compile()
    _CACHE[variant] = nc
    return nc


def make_in_maps(x, inputs_mask, Wq, bq, Wk, bk, Wv, bv, Wo, bo,
                 sorted_mode=True):
    """Host-side shard prep. All args np.float32/int32 full tensors.

    sorted_mode: per batch, tokens are permuted so mask=1 tokens come first
    (attention is permutation-equivariant when q/k/v share the permutation);
    returns the per-batch permutations for un-permuting the output.
    """
    in_maps = []
    m_all = inputs_mask.astype(np.float32)
    perms = []
    for b in range(B):
        if sorted_mode:
            perms.append(np.argsort(-m_all[b], kind="stable"))
        else:
            perms.append(np.arange(N))
    for c in range(NC_):
        b, hh = c // 2, c % 2
        cs = slice(hh * FH, (hh + 1) * FH)
        m = m_all[b][perms[b]]
        xb = x[b][perms[b]]
        im = {
            "xT": np.ascontiguousarray(xb.T).astype(NPBF16),
            "wq": Wq[:, cs].astype(NPBF16),
            "wk": Wk[:, cs].astype(NPBF16),
            "wv": Wv[:, cs].astype(NPBF16),
            "wo": np.ascontiguousarray(Wo[cs, :]).astype(NPBF16),
            "bqk": np.stack(
                [bq[cs].reshape(4, 128), bk[cs].reshape(4, 128)], axis=0
            ).reshape(8, 128).T.astype(np.float32).copy(),
            "bvb": np.broadcast_to(bv[cs], (128, FH)).astype(np.float32).copy(),
            "mjb": m.reshape(NJ, 128).T.astype(np.float32).copy(),
            "emn": np.exp(-m).reshape(NTOKC, 128).T.astype(np.float32).copy(),
            "mr4": np.stack([m, 1.0 - m, m, 1.0 - m]).astype(np.float32).copy(),
            "mrp": np.stack([m, m, 1.0 - m, 1.0 - m]).astype(np.float32).copy(),
        }
        in_maps.append(im)
    return in_maps, perms


def kernel(x, inputs_mask, Wq, bq, Wk, bk, Wv, bv, Wo, bo):
    x = np.asarray(x, dtype=np.float32)
    inputs_mask = np.asarray(inputs_mask)
    Wq, bq = np.asarray(Wq, np.float32), np.asarray(bq, np.float32)
    Wk, bk = np.asarray(Wk, np.float32), np.asarray(bk, np.float32)
    Wv, bv = np.asarray(Wv, np.float32), np.asarray(bv, np.float32)
    Wo, bo = np.asarray(Wo, np.float32), np.asarray(bo, np.float32)

    # sorted mode requires the mask-1 count per batch to land inside query
    # blocks 1..2 (always true for ~Bernoulli(0.5) masks); fall back to the
    # static dual-pass program otherwise.
    c1 = inputs_mask.astype(np.int64).sum(axis=1)
    sorted_mode = bool(np.all((c1 >= 512) & (c1 <= 3 * 512)))
    nc = build_program("sorted" if sorted_mode else "dual")
    in_maps, perms = make_in_maps(
        x, inputs_mask, Wq, bq, Wk, bk, Wv, bv, Wo, bo, sorted_mode=sorted_mode
    )
    res = bass_utils.run_bass_kernel_spmd(nc, in_maps, core_ids=list(range(NC_)))
    out = np.empty((B, N, F), dtype=np.float32)
    for b in range(B):
        out[b][perms[b]] = (
            res.results[2 * b]["y"] + res.results[2 * b + 1]["y"] + bo
        )
    return out
